# revision 31
# baseline (speedup 1.0000x reference)
"""Cross-temporal attention Trainium2 (Bass/Tile) kernel.

Problem: two streams x1, x2 of shape [B=4, C=256, H=64, W=64]; tokens are the
H*W=4096 spatial positions. Per batch b and stream s:
    q_s = t_s @ Wq.T + bq ; k_s = t_s @ Wk.T + bk ; v_s = t_s @ Wv.T + bv
    out_s = softmax(q_s @ k_{3-s}.T) @ v_s            (no 1/sqrt(d) scaling)

Sharding: 8 NeuronCores, one (batch, stream) unit per core (4 batches x 2
streams). Fully SPMD — the same program runs on every core, only the input
bindings differ. No collectives.

Math refactor (v2): softmax is invariant to per-query logit shifts, so
    q1 @ k2.T  =  t1 (Wq.T Wk) t2.T + [t1 Wq.T bk]_n + [bq.Wk t2]_m + bq.bk
drops its per-query terms. With Wqk = Wq.T@Wk precomputed on host:
    ST_block = Xb_block^T @ ZT,   ZT = Wqk^T @ Xa    (NO K projection at all;
raw xb chunks are the S-matmul weights), and the per-KEY bias
    w[m] = (Wk^T bq)·Xb[:,m] + bq·bk
is folded into the exp as the activation's per-partition bias — zero extra ops.

Dtypes: x / ZT / V are fp16 (host converts x; halves DMA + SBUF, enables FWL
weight loads, exact fp22 upconvert in the PE). E = exp(S + w) is bf16 with NO
max subtraction: logits are <~32 so e^s < 5e13, and bf16's f32-like exponent
range absorbs the full per-query spread (fp16/fp8 would underflow weak
queries). exp noise ~0.2% cancels between numerator U and denominator D.
Measured rel err ~3e-3 (CPU sim + HW), budget 2e-2.

Per-core layout (all transposed, [C,N] channel-major — zero transposes):
    ZT = Wqk^T Xa  [C, N] fp16
    V  = Xa^T Wv^T + bv  [N, C] fp16
    ST = Xb-block^T @ ZT = scores^T  [m, n] blocks  (softmax over m)
    E  = exp(ST + w - 0)  bf16
    U  = accum_m V-block^T @ E  -> [C, n]  (psum f32)
    D  = column sums of E (DVE accumulate + ones-matmul replicate)
    OT = U / D  [C, N]

Perf notes (evolved from v1's 355->290us; v2 targets ~260us):
 - attention uses 1-bank [128,512] score psum tiles (bufs=4 pipeline), one exp
   per 512-half, one dacc add per key block covering the full 1024-wide pair.
 - each pair's normalize/store tail is deferred into the NEXT pair's stream.
 - HAM warmup: dependency-free matmuls on zeroed tiles bridge the initial
   DMA window so the PE clock gate arms (K=8/8) before real work.
 - projections (now only ZT + V, KT is gone) are dissolved into pair 0's
   key-block stream just-in-time; the PE always has dense 512-free work.
 - weights pre-swizzled on host; input DMAs ordered by first consumption.
 - reciprocal_approx_fast (18 bits) for the softmax denominators.
"""

import numpy as np

import concourse.bacc as bacc
import concourse.mybir as mybir
import concourse.tile as tile
from concourse.bass_utils import run_bass_kernel_spmd

F32 = mybir.dt.float32
F32R = mybir.dt.float32r
F16 = mybir.dt.float16
BF16 = mybir.dt.bfloat16
AF = mybir.ActivationFunctionType

B, C, H, W = 4, 256, 64, 64
N = H * W            # 4096 tokens
CK = C // 128        # 2 channel chunks of 128
NT = 512             # attention n-tile (query block, free dim)
NP = 1024            # n-tile pair width
N_PAIR = N // NP     # 4
MB = 128             # key/value block (partition block)
N_MB = N // MB       # 32
MB_PER_PIECE = NP // MB   # 8 key blocks per piece
SKEW = 3             # software-pipeline skew between S and U matmuls

_NC_CACHE = None
LAST_RESULT = None   # BassKernelResults of the most recent kernel() call


def _build():
    nc = bacc.Bacc("TRN2", target_bir_lowering=False, debug=False)

    xa = nc.dram_tensor("xa", [C, N], F16, kind="ExternalInput").ap()
    xb = nc.dram_tensor("xb", [C, N], F16, kind="ExternalInput").ap()
    # single packed parameter blob (one DMA): per partition, fp16 columns:
    #   [0:512)      wqk lhsT  (flat ki*C + m)
    #   [512:1024)   wv lhsT
    #   [1024:1088)  wb per-key exp bias, f32 pairs (32 values)
    #   [1088:1600)  bv replicated to all partitions, f32 pairs (256 values)
    PCOLS = 1600
    params_d = nc.dram_tensor("params", [128, PCOLS], F16,
                              kind="ExternalInput").ap()
    out = nc.dram_tensor("o", [C, N], F32, kind="ExternalOutput").ap()

    with tile.TileContext(nc) as tc:
        with tc.tile_pool(name="persist", bufs=1) as pp, \
             tc.tile_pool(name="os", bufs=4) as op_, \
             tc.tile_pool(name="s_ps", bufs=4, space="PSUM") as sp, \
             tc.tile_pool(name="u_ps", bufs=1, space="PSUM") as up, \
             tc.tile_pool(name="e_sb", bufs=5) as ep, \
             tc.tile_pool(name="acc", bufs=2) as ap_:
            # ---- HAM warmup (emitted first, zero data deps) -----------
            warm_w = pp.tile([128, 128], F32R, tag="warm_w")
            warm_src = pp.tile([128, NT], F32R, tag="warm_src")
            nc.vector.memset(warm_w[:].bitcast(F32), 0.0)
            nc.vector.memset(warm_src[:].bitcast(F32), 0.0)
            warm_ps = sp.tile([128, NT], F32, tag="s")
            N_WARM = 14
            for it in range(N_WARM):
                nc.tensor.matmul(warm_ps[:], warm_w[:], warm_src[:],
                                 start=(it == 0), stop=(it == N_WARM - 1))

            # ---- parameters & inputs, in consumption order ------------
            params = pp.tile([128, PCOLS], F16, tag="params")

            def wqk_sl(ki, co):
                return params[:, ki * C + co * 128:ki * C + (co + 1) * 128]

            def wv_sl(ki):
                return params[:, 512 + ki * C:512 + (ki + 1) * C]

            def wb_sl(mb):
                return params[:, 1024 + 2 * mb:1024 + 2 * mb + 2].bitcast(F32)

            bv_rep = params[:, 1088:1600].bitcast(F32).rearrange(
                "p (k j) -> p k j", k=CK)

            xa_pieces = {}
            for pc in range(4):
                for ki in range(CK):
                    xa_pieces[(ki, pc)] = pp.tile(
                        [128, NP], F16, name=f"xa_{ki}_{pc}", tag=f"xa_{ki}_{pc}")

            def xa_rhs(ki, nt):
                # 512-wide rhs slice of xa for ZT tile nt
                piece = xa_pieces[(ki, nt // 2)]
                return piece[:, (nt % 2) * NT:((nt % 2) + 1) * NT]

            def xa_mb(ki, mb):
                # 128-wide lhsT slice of xa for V block mb
                piece = xa_pieces[(ki, mb // MB_PER_PIECE)]
                off = (mb % MB_PER_PIECE) * 128
                return piece[:, off:off + 128]

            # xb pieces are the S-matmul weights for every pair — persistent
            xb_pieces = {}
            for pc in range(4):
                for ki in range(CK):
                    xb_pieces[(ki, pc)] = pp.tile(
                        [128, NP], F16, name=f"xb_{ki}_{pc}", tag=f"xb_{ki}_{pc}")

            def xb_mb(ki, mb):
                # 128-wide lhsT slice of xb for S block mb (key block)
                piece = xb_pieces[(ki, mb // MB_PER_PIECE)]
                off = (mb % MB_PER_PIECE) * 128
                return piece[:, off:off + 128]

            def dma_x(pieces, src, ki, pc):
                nc.sync.dma_start(
                    pieces[(ki, pc)][:],
                    src[ki * 128:(ki + 1) * 128, pc * NP:(pc + 1) * NP])

            # DMA queue is serial (~420 GB/s, ~0.3us/instr overhead): few big
            # transfers, ordered strictly by first need
            nc.sync.dma_start(params[:], params_d)
            dma_x(xa_pieces, xa, 0, 0)
            dma_x(xa_pieces, xa, 1, 0)
            dma_x(xb_pieces, xb, 0, 0)
            dma_x(xb_pieces, xb, 1, 0)
            for pc in range(1, 4):
                dma_x(xa_pieces, xa, 0, pc)
                dma_x(xa_pieces, xa, 1, pc)
                dma_x(xb_pieces, xb, 0, pc)
                dma_x(xb_pieces, xb, 1, pc)

            ones_f = pp.tile([128, 128], F32, tag="ones_f")
            nc.vector.memset(ones_f[:], 1.0)
            ones_r = pp.tile([128, 128], F32R, tag="ones_r")
            nc.vector.tensor_copy(ones_r[:], ones_f[:])
            ones_b = pp.tile([128, 128], BF16, tag="ones_b")
            nc.vector.tensor_copy(ones_b[:], ones_f[:])

            # per-pair piece tiles for ZT / V
            zt_p = [pp.tile([128, CK, NP], F16, name=f"zt_{p}", tag=f"zt_{p}")
                    for p in range(N_PAIR)]
            v_p = [pp.tile([128, CK, NP], F16, name=f"v_{p}", tag=f"v_{p}")
                   for p in range(N_PAIR)]

            # ---- projection emitters ---------------------------------
            def emit_zt(co, nt):
                ps = sp.tile([128, NT], F32, tag="s")
                half = ps[:]
                for ki in range(CK):
                    nc.tensor.matmul(
                        half, wqk_sl(ki, co),
                        xa_rhs(ki, nt), start=(ki == 0), stop=(ki == CK - 1))
                nc.vector.tensor_copy(
                    zt_p[nt // 2][:, co, (nt % 2) * NT:((nt % 2) + 1) * NT],
                    half)

            def emit_v(mb):
                ps = sp.tile([128, NT], F32, tag="s")
                half = ps[:, 0:C]
                for ki in range(CK):
                    nc.tensor.matmul(
                        half, xa_mb(ki, mb), wv_sl(ki),
                        start=(ki == 0), stop=(ki == CK - 1))
                off = (mb % MB_PER_PIECE) * 128
                nc.vector.tensor_add(
                    v_p[mb // MB_PER_PIECE][:, :, off:off + 128],
                    half.rearrange("p (c j) -> p c j", c=CK), bv_rep)

            # ---- attention emitters ----------------------------------
            # Query range is processed in segments: three 1024-wide + two
            # 512-wide at the end. The narrow final segments halve the
            # serial normalize/store tail after the very last U matmul.
            SEGS = [(0, 0, NP), (1, NP, NP), (2, 2 * NP, NP),
                    (3, 3 * NP, NT), (4, 3 * NP + NT, NT)]
            N_SEG = len(SEGS)
            seg_state = {}

            def zt_slice(base, ho, ki):
                col = base + ho * NT
                return zt_p[col // NP][:, ki, col % NP:col % NP + NT]

            lastu = {}

            def attn_begin(sg):
                _, base, width = SEGS[sg]
                n_ho = width // NT
                if width == NP:
                    ut = [up.tile([128, NP], F32, name=f"u_{sg}_{co}",
                                  tag=f"u{co}") for co in range(CK)]
                    uoff = 0
                else:
                    # the two final 512-wide segments share one PSUM tile per
                    # co, in disjoint column halves — no write-after-read
                    # stall at the segment boundary
                    if not lastu:
                        for co in range(CK):
                            lastu[co] = up.tile([128, NP], F32,
                                                name=f"u_last_{co}",
                                                tag=f"u{co}")
                    ut = [lastu[co] for co in range(CK)]
                    uoff = ((base // NT) % 2) * NT
                seg_state[sg] = {
                    "ut": ut, "uoff": uoff,
                    "dacc": ap_.tile([128, width], F32R, name=f"dacc_{sg}",
                                     tag=f"dacc_w{width}"),
                    "e": {},
                    "base": base, "width": width, "n_ho": n_ho,
                }

            def attn_step(sg, step):
                st = seg_state[sg]
                base, width, n_ho = st["base"], st["width"], st["n_ho"]
                last = sg == N_SEG - 1
                if step < N_MB:
                    mb = step
                    s_h = [sp.tile([128, NT], F32, name=f"s_h{ho}", tag="s")
                           for ho in range(n_ho)]
                    for ki in range(CK):
                        for ho in range(n_ho):
                            nc.tensor.matmul(
                                s_h[ho][:],
                                xb_mb(ki, mb),
                                zt_slice(base, ho, ki),
                                start=(ki == 0), stop=(ki == CK - 1))
                    e_r = ep.tile([128, width], BF16, tag=f"e_w{width}")
                    for ho in range(n_ho):
                        nc.scalar.activation(
                            e_r[:, ho * NT:(ho + 1) * NT], s_h[ho][:], AF.Exp,
                            bias=wb_sl(mb))
                    st["e"][mb] = e_r
                    if last and mb == N_MB - 1:
                        st["e_last"] = e_r  # D takes this block via matmul
                if step >= SKEW:
                    mb = step - SKEW
                    e_r = st["e"].pop(mb)
                    vp = v_p[mb // MB_PER_PIECE]
                    off = (mb % MB_PER_PIECE) * 128
                    uoff = st["uoff"]
                    for co in range(CK):
                        for ho in range(n_ho):
                            nc.tensor.matmul(
                                st["ut"][co][:, uoff + ho * NT:
                                             uoff + (ho + 1) * NT],
                                vp[:, co, off:off + 128],
                                e_r[:, ho * NT:(ho + 1) * NT],
                                start=(mb == 0), stop=(mb == N_MB - 1))
                    if mb == 0:
                        nc.vector.tensor_copy(st["dacc"][:], e_r[:])
                    elif not (last and mb == N_MB - 1):
                        nc.vector.tensor_add(st["dacc"][:], st["dacc"][:], e_r[:])

            def emit_d(sg):
                # D = colsum of E, replicated to all partitions via ones-matmul;
                # for the last segment the final E block rides in as a second
                # accumulated matmul (its DVE dacc add would sit on the
                # critical tail) and D/recip overlap the last U matmuls.
                st = seg_state[sg]
                width, n_ho = st["width"], st["n_ho"]
                dinv = ap_.tile([128, width], F32, name=f"dinv_{sg}",
                                tag=f"dinv_w{width}")
                e_last = st.pop("e_last", None)
                for ho in range(n_ho):
                    d_ps = sp.tile([128, NT], F32, name=f"d_{ho}", tag="s")
                    nc.tensor.matmul(d_ps[:], ones_r[:],
                                     st["dacc"][:, ho * NT:(ho + 1) * NT],
                                     start=True, stop=(e_last is None))
                    if e_last is not None:
                        nc.tensor.matmul(d_ps[:], ones_b[:],
                                         e_last[:, ho * NT:(ho + 1) * NT],
                                         start=False, stop=True)
                    nc.vector.reciprocal_approx_fast(
                        dinv[:, ho * NT:(ho + 1) * NT], d_ps[:])
                st["dinv"] = dinv

            def attn_end(sg):
                # per-512-half normalize + store so the output DMA of one half
                # overlaps the multiply of the next (shrinks the serial tail)
                st = seg_state.pop(sg)
                dinv = st["dinv"]
                base, n_ho, uoff = st["base"], st["n_ho"], st["uoff"]
                for co in range(CK):
                    for ho in range(n_ho):
                        o_sb = op_.tile([128, NT], F32, tag="o_sb")
                        nc.vector.tensor_mul(
                            o_sb[:], st["ut"][co][:, uoff + ho * NT:
                                                  uoff + (ho + 1) * NT],
                            dinv[:, ho * NT:(ho + 1) * NT])
                        nc.sync.dma_start(
                            out[co * 128:(co + 1) * 128,
                                base + ho * NT:base + (ho + 1) * NT],
                            o_sb[:])

            # ---- emission schedule -----------------------------------
            # Projection work is distributed just-in-time through pair 0's
            # key-block stream (standalone projection phases run at low PE
            # duty and HAM re-throttles). Only pair-0 prerequisites up front.
            for co in range(CK):
                for nto in range(2):
                    emit_zt(co, nto)
            for mb in range(4):     # needs only params + xa piece0; bridges
                emit_v(mb)          # the PE over the xb piece0 DMA wait

            # just-in-time jobs sprinkled through pair 0's steps
            extra = {}

            def add_extra(step, fn):
                extra.setdefault(step, []).append(fn)

            for mb in range(4, 8):          # remaining early v blocks
                add_extra(1 + (mb - 4) // 2, lambda mb=mb: emit_v(mb))
            for mb in range(8, N_MB):       # v block 4 steps ahead of its U
                add_extra(mb - 4, lambda mb=mb: emit_v(mb))
            for p in range(1, 4):           # zt pieces 1-3 anywhere in pair 0
                for i, (co, nto) in enumerate(
                        ((0, 0), (0, 1), (1, 0), (1, 1))):
                    add_extra(1 + p * 7 + i,
                              lambda co=co, nt=2 * p + nto: emit_zt(co, nt))

            attn_begin(0)
            for step in range(N_MB + SKEW):
                attn_step(0, step)
                for fn in extra.pop(step, ()):
                    fn()
            for sg in range(1, N_SEG):
                attn_begin(sg)
                for step in range(N_MB + SKEW):
                    attn_step(sg, step)
                    if step == 1:
                        emit_d(sg - 1)
                    if step == 2:
                        attn_end(sg - 1)
                    if sg == N_SEG - 1 and step == N_MB + 1:
                        # dacc complete (mb30 add emitted at this step; mb31
                        # rides in via matmul) — D/recip overlap final U MMs
                        emit_d(sg)
            attn_end(N_SEG - 1)
    nc.compile()
    return nc


def _get_nc():
    global _NC_CACHE
    if _NC_CACHE is None:
        _NC_CACHE = _build()
    return _NC_CACHE


def _w_layout(w, dtype=np.float16):
    # lhsT chunks for "x @ w.T": w_l[p, ki*C + m] = w.T[ki*128 + p, m]
    wt = np.ascontiguousarray(np.asarray(w, np.float32).T)      # [C_in, C_out]
    return np.ascontiguousarray(
        wt.reshape(CK, 128, C).transpose(1, 0, 2).reshape(128, CK * C)
    ).astype(dtype)


def kernel(x1, x2, Wq, bq, Wk, bk, Wv, bv):
    global LAST_RESULT
    x1 = np.asarray(x1, dtype=np.float32)
    x2 = np.asarray(x2, dtype=np.float32)
    Wq = np.asarray(Wq, np.float32)
    Wk = np.asarray(Wk, np.float32)
    bq = np.asarray(bq, np.float32)
    bk = np.asarray(bk, np.float32)

    Wqk = (Wq.T.astype(np.float64) @ Wk.astype(np.float64)).astype(np.float32)
    wk_bq = (Wk.T.astype(np.float64) @ bq.astype(np.float64)).astype(np.float32)
    const = float(np.dot(bq.astype(np.float64), bk.astype(np.float64)))

    x1h = x1.reshape(B, C, N).astype(np.float16)
    x2h = x2.reshape(B, C, N).astype(np.float16)

    wqk_l = _w_layout(Wqk.T)                # ZT = Wqk^T @ X  <=>  z = t @ Wqk
    wv_l = _w_layout(Wv)
    bv_rep = np.tile(np.asarray(bv, np.float32).reshape(1, C), (128, 1))
    in_maps = []
    for core in range(8):
        b, s = core % B, core // B
        xsh, xoh = (x1h, x2h) if s == 0 else (x2h, x1h)
        xb_f32 = xoh[b].astype(np.float32)
        wvec = wk_bq @ xb_f32 + const                      # [N] per key
        wb_l = np.ascontiguousarray(wvec.reshape(N_MB, 128).T)  # [128, 32] f32
        blob = np.concatenate([
            wqk_l.view(np.uint8), wv_l.view(np.uint8),
            wb_l.view(np.uint8), bv_rep.view(np.uint8),
        ], axis=1)
        in_maps.append({
            "xa": np.ascontiguousarray(xsh[b]),
            "xb": np.ascontiguousarray(xoh[b]),
            "params": blob.view(np.float16),
        })
    nc = _get_nc()
    res = run_bass_kernel_spmd(nc, in_maps, list(range(8)))
    LAST_RESULT = res
    x1_out = np.stack([res.results[b]["o"].reshape(C, H, W) for b in range(B)])
    x2_out = np.stack([res.results[B + b]["o"].reshape(C, H, W) for b in range(B)])
    return (x1_out, x2_out)


# revision 32
# speedup vs baseline: 1.0098x; 1.0098x over previous
"""Cross-temporal attention Trainium2 (Bass/Tile) kernel.

Problem: two streams x1, x2 of shape [B=4, C=256, H=64, W=64]; tokens are the
H*W=4096 spatial positions. Per batch b and stream s:
    q_s = t_s @ Wq.T + bq ; k_s = t_s @ Wk.T + bk ; v_s = t_s @ Wv.T + bv
    out_s = softmax(q_s @ k_{3-s}.T) @ v_s            (no 1/sqrt(d) scaling)

Sharding: 8 NeuronCores, one (batch, stream) unit per core (4 batches x 2
streams). Fully SPMD — the same program runs on every core, only the input
bindings differ. No collectives.

Math refactor (v2): softmax is invariant to per-query logit shifts, so
    q1 @ k2.T  =  t1 (Wq.T Wk) t2.T + [t1 Wq.T bk]_n + [bq.Wk t2]_m + bq.bk
drops its per-query terms. With Wqk = Wq.T@Wk precomputed on host:
    ST_block = Xb_block^T @ ZT,   ZT = Wqk^T @ Xa    (NO K projection at all;
raw xb chunks are the S-matmul weights), and the per-KEY bias
    w[m] = (Wk^T bq)·Xb[:,m] + bq·bk
is folded into the exp as the activation's per-partition bias — zero extra ops.

Dtypes: x / ZT / V are fp16 (host converts x; halves DMA + SBUF, enables FWL
weight loads, exact fp22 upconvert in the PE). E = exp(S + w) is bf16 with NO
max subtraction: logits are <~32 so e^s < 5e13, and bf16's f32-like exponent
range absorbs the full per-query spread (fp16/fp8 would underflow weak
queries). exp noise ~0.2% cancels between numerator U and denominator D.
Measured rel err ~3e-3 (CPU sim + HW), budget 2e-2.

Per-core layout (all transposed, [C,N] channel-major — zero transposes):
    ZT = Wqk^T Xa  [C, N] fp16
    V  = Xa^T Wv^T + bv  [N, C] fp16
    ST = Xb-block^T @ ZT = scores^T  [m, n] blocks  (softmax over m)
    E  = exp(ST + w - 0)  bf16
    U  = accum_m V-block^T @ E  -> [C, n]  (psum f32)
    D  = column sums of E (DVE accumulate + ones-matmul replicate)
    OT = U / D  [C, N]

Perf notes (evolved from v1's 355->290us; v2 targets ~260us):
 - attention uses 1-bank [128,512] score psum tiles (bufs=4 pipeline), one exp
   per 512-half, one dacc add per key block covering the full 1024-wide pair.
 - each pair's normalize/store tail is deferred into the NEXT pair's stream.
 - HAM warmup: dependency-free matmuls on zeroed tiles bridge the initial
   DMA window so the PE clock gate arms (K=8/8) before real work.
 - projections (now only ZT + V, KT is gone) are dissolved into pair 0's
   key-block stream just-in-time; the PE always has dense 512-free work.
 - weights pre-swizzled on host; input DMAs ordered by first consumption.
 - reciprocal_approx_fast (18 bits) for the softmax denominators.
"""

import numpy as np

import concourse.bacc as bacc
import concourse.mybir as mybir
import concourse.tile as tile
from concourse.bass_utils import run_bass_kernel_spmd

F32 = mybir.dt.float32
F32R = mybir.dt.float32r
F16 = mybir.dt.float16
BF16 = mybir.dt.bfloat16
AF = mybir.ActivationFunctionType

B, C, H, W = 4, 256, 64, 64
N = H * W            # 4096 tokens
CK = C // 128        # 2 channel chunks of 128
NT = 512             # attention n-tile (query block, free dim)
NP = 1024            # n-tile pair width
N_PAIR = N // NP     # 4
MB = 128             # key/value block (partition block)
N_MB = N // MB       # 32
MB_PER_PIECE = NP // MB   # 8 key blocks per piece
SKEW = 3             # software-pipeline skew between S and U matmuls

_NC_CACHE = None
LAST_RESULT = None   # BassKernelResults of the most recent kernel() call


def _build():
    nc = bacc.Bacc("TRN2", target_bir_lowering=False, debug=False)

    xa = nc.dram_tensor("xa", [C, N], F16, kind="ExternalInput").ap()
    xb = nc.dram_tensor("xb", [C, N], F16, kind="ExternalInput").ap()
    # single packed parameter blob (one DMA): per partition, fp16 columns:
    #   [0:512)      wqk lhsT  (flat ki*C + m)
    #   [512:1024)   wv lhsT
    #   [1024:1088)  wb per-key exp bias, f32 pairs (32 values)
    #   [1088:1600)  bv replicated to all partitions, f32 pairs (256 values)
    PCOLS = 1600
    params_d = nc.dram_tensor("params", [128, PCOLS], F16,
                              kind="ExternalInput").ap()
    out = nc.dram_tensor("o", [C, N], F32, kind="ExternalOutput").ap()

    with tile.TileContext(nc) as tc:
        with tc.tile_pool(name="persist", bufs=1) as pp, \
             tc.tile_pool(name="os", bufs=4) as op_, \
             tc.tile_pool(name="s_ps", bufs=4, space="PSUM") as sp, \
             tc.tile_pool(name="u_ps", bufs=1, space="PSUM") as up, \
             tc.tile_pool(name="e_sb", bufs=5) as ep, \
             tc.tile_pool(name="acc", bufs=2) as ap_:
            # ---- HAM warmup (emitted first, zero data deps) -----------
            warm_w = pp.tile([128, 128], F32R, tag="warm_w")
            warm_src = pp.tile([128, NT], F32R, tag="warm_src")
            nc.vector.memset(warm_w[:].bitcast(F32), 0.0)
            nc.vector.memset(warm_src[:].bitcast(F32), 0.0)
            warm_ps = sp.tile([128, NT], F32, tag="s")
            N_WARM = 14
            for it in range(N_WARM):
                nc.tensor.matmul(warm_ps[:], warm_w[:], warm_src[:],
                                 start=(it == 0), stop=(it == N_WARM - 1))

            # ---- parameters & inputs, in consumption order ------------
            params = pp.tile([128, PCOLS], F16, tag="params")

            def wqk_sl(ki, co):
                return params[:, ki * C + co * 128:ki * C + (co + 1) * 128]

            def wv_sl(ki):
                return params[:, 512 + ki * C:512 + (ki + 1) * C]

            def wb_sl(mb):
                return params[:, 1024 + 2 * mb:1024 + 2 * mb + 2].bitcast(F32)

            bv_rep = params[:, 1088:1600].bitcast(F32).rearrange(
                "p (k j) -> p k j", k=CK)

            xa_pieces = {}
            for pc in range(4):
                for ki in range(CK):
                    xa_pieces[(ki, pc)] = pp.tile(
                        [128, NP], F16, name=f"xa_{ki}_{pc}", tag=f"xa_{ki}_{pc}")

            def xa_rhs(ki, nt):
                # 512-wide rhs slice of xa for ZT tile nt
                piece = xa_pieces[(ki, nt // 2)]
                return piece[:, (nt % 2) * NT:((nt % 2) + 1) * NT]

            def xa_mb(ki, mb):
                # 128-wide lhsT slice of xa for V block mb
                piece = xa_pieces[(ki, mb // MB_PER_PIECE)]
                off = (mb % MB_PER_PIECE) * 128
                return piece[:, off:off + 128]

            # xb pieces are the S-matmul weights for every pair — persistent
            xb_pieces = {}
            for pc in range(4):
                for ki in range(CK):
                    xb_pieces[(ki, pc)] = pp.tile(
                        [128, NP], F16, name=f"xb_{ki}_{pc}", tag=f"xb_{ki}_{pc}")

            def xb_mb(ki, mb):
                # 128-wide lhsT slice of xb for S block mb (key block)
                piece = xb_pieces[(ki, mb // MB_PER_PIECE)]
                off = (mb % MB_PER_PIECE) * 128
                return piece[:, off:off + 128]

            def dma_x(pieces, src, ki, pc):
                nc.sync.dma_start(
                    pieces[(ki, pc)][:],
                    src[ki * 128:(ki + 1) * 128, pc * NP:(pc + 1) * NP])

            # DMA queue is serial (~420 GB/s, ~0.3us/instr overhead): few big
            # transfers, ordered strictly by first need
            nc.sync.dma_start(params[:], params_d)
            dma_x(xa_pieces, xa, 0, 0)
            dma_x(xa_pieces, xa, 1, 0)
            dma_x(xb_pieces, xb, 0, 0)
            dma_x(xb_pieces, xb, 1, 0)
            for pc in range(1, 4):
                dma_x(xa_pieces, xa, 0, pc)
                dma_x(xa_pieces, xa, 1, pc)
                dma_x(xb_pieces, xb, 0, pc)
                dma_x(xb_pieces, xb, 1, pc)

            ones_f = pp.tile([128, 128], F32, tag="ones_f")
            nc.vector.memset(ones_f[:], 1.0)
            ones_r = pp.tile([128, 128], F32R, tag="ones_r")
            nc.vector.tensor_copy(ones_r[:], ones_f[:])
            ones_b = pp.tile([128, 128], BF16, tag="ones_b")
            nc.vector.tensor_copy(ones_b[:], ones_f[:])

            # per-pair piece tiles for ZT / V
            zt_p = [pp.tile([128, CK, NP], F16, name=f"zt_{p}", tag=f"zt_{p}")
                    for p in range(N_PAIR)]
            v_p = [pp.tile([128, CK, NP], F16, name=f"v_{p}", tag=f"v_{p}")
                   for p in range(N_PAIR)]

            # ---- projection emitters ---------------------------------
            def emit_zt(co, nt):
                ps = sp.tile([128, NT], F32, tag="s")
                half = ps[:]
                for ki in range(CK):
                    nc.tensor.matmul(
                        half, wqk_sl(ki, co),
                        xa_rhs(ki, nt), start=(ki == 0), stop=(ki == CK - 1))
                nc.vector.tensor_copy(
                    zt_p[nt // 2][:, co, (nt % 2) * NT:((nt % 2) + 1) * NT],
                    half)

            def emit_v(mb):
                ps = sp.tile([128, NT], F32, tag="s")
                half = ps[:, 0:C]
                for ki in range(CK):
                    nc.tensor.matmul(
                        half, xa_mb(ki, mb), wv_sl(ki),
                        start=(ki == 0), stop=(ki == CK - 1))
                off = (mb % MB_PER_PIECE) * 128
                nc.vector.tensor_add(
                    v_p[mb // MB_PER_PIECE][:, :, off:off + 128],
                    half.rearrange("p (c j) -> p c j", c=CK), bv_rep)

            # ---- attention emitters ----------------------------------
            # Query range is processed in segments: three 1024-wide + two
            # 512-wide at the end. The narrow final segments halve the
            # serial normalize/store tail after the very last U matmul.
            SEGS = [(0, 0, NP), (1, NP, NP), (2, 2 * NP, NP),
                    (3, 3 * NP, NT), (4, 3 * NP + NT, NT)]
            N_SEG = len(SEGS)
            seg_state = {}

            def zt_slice(base, ho, ki):
                col = base + ho * NT
                return zt_p[col // NP][:, ki, col % NP:col % NP + NT]

            lastu = {}

            def attn_begin(sg):
                _, base, width = SEGS[sg]
                n_ho = width // NT
                if width == NP:
                    ut = [up.tile([128, NP], F32, name=f"u_{sg}_{co}",
                                  tag=f"u{co}") for co in range(CK)]
                    uoff = 0
                else:
                    # the two final 512-wide segments share one PSUM tile per
                    # co, in disjoint column halves — no write-after-read
                    # stall at the segment boundary
                    if not lastu:
                        for co in range(CK):
                            lastu[co] = up.tile([128, NP], F32,
                                                name=f"u_last_{co}",
                                                tag=f"u{co}")
                    ut = [lastu[co] for co in range(CK)]
                    uoff = ((base // NT) % 2) * NT
                seg_state[sg] = {
                    "ut": ut, "uoff": uoff,
                    "dacc": ap_.tile([128, width], F32R, name=f"dacc_{sg}",
                                     tag=f"dacc_w{width}"),
                    "e": {},
                    "base": base, "width": width, "n_ho": n_ho,
                }

            def attn_step(sg, step):
                st = seg_state[sg]
                base, width, n_ho = st["base"], st["width"], st["n_ho"]
                last = sg == N_SEG - 1
                if step < N_MB:
                    mb = step
                    s_h = [sp.tile([128, NT], F32, name=f"s_h{ho}", tag="s")
                           for ho in range(n_ho)]
                    for ki in range(CK):
                        for ho in range(n_ho):
                            nc.tensor.matmul(
                                s_h[ho][:],
                                xb_mb(ki, mb),
                                zt_slice(base, ho, ki),
                                start=(ki == 0), stop=(ki == CK - 1))
                    e_r = ep.tile([128, width], BF16, tag=f"e_w{width}")
                    for ho in range(n_ho):
                        nc.scalar.activation(
                            e_r[:, ho * NT:(ho + 1) * NT], s_h[ho][:], AF.Exp,
                            bias=wb_sl(mb))
                    st["e"][mb] = e_r
                    if last and mb == N_MB - 1:
                        st["e_last"] = e_r  # D takes this block via matmul
                if step >= SKEW:
                    mb = step - SKEW
                    e_r = st["e"].pop(mb)
                    vp = v_p[mb // MB_PER_PIECE]
                    off = (mb % MB_PER_PIECE) * 128
                    uoff = st["uoff"]
                    for co in range(CK):
                        for ho in range(n_ho):
                            nc.tensor.matmul(
                                st["ut"][co][:, uoff + ho * NT:
                                             uoff + (ho + 1) * NT],
                                vp[:, co, off:off + 128],
                                e_r[:, ho * NT:(ho + 1) * NT],
                                start=(mb == 0), stop=(mb == N_MB - 1))
                    if mb == 0:
                        nc.vector.tensor_copy(st["dacc"][:], e_r[:])
                    elif not (last and mb == N_MB - 1):
                        nc.vector.tensor_add(st["dacc"][:], st["dacc"][:], e_r[:])

            def emit_d(sg):
                # D = colsum of E, replicated to all partitions via ones-matmul;
                # for the last segment the final E block rides in as a second
                # accumulated matmul (its DVE dacc add would sit on the
                # critical tail) and D/recip overlap the last U matmuls.
                st = seg_state[sg]
                width, n_ho = st["width"], st["n_ho"]
                dinv = ap_.tile([128, width], F32, name=f"dinv_{sg}",
                                tag=f"dinv_w{width}")
                e_last = st.pop("e_last", None)
                for ho in range(n_ho):
                    d_ps = sp.tile([128, NT], F32, name=f"d_{ho}", tag="s")
                    nc.tensor.matmul(d_ps[:], ones_r[:],
                                     st["dacc"][:, ho * NT:(ho + 1) * NT],
                                     start=True, stop=(e_last is None))
                    if e_last is not None:
                        nc.tensor.matmul(d_ps[:], ones_b[:],
                                         e_last[:, ho * NT:(ho + 1) * NT],
                                         start=False, stop=True)
                    nc.vector.reciprocal_approx_fast(
                        dinv[:, ho * NT:(ho + 1) * NT], d_ps[:])
                st["dinv"] = dinv

            def attn_end(sg):
                # per-512-half normalize + store so the output DMA of one half
                # overlaps the multiply of the next (shrinks the serial tail)
                st = seg_state.pop(sg)
                dinv = st["dinv"]
                base, n_ho, uoff = st["base"], st["n_ho"], st["uoff"]
                for co in range(CK):
                    for ho in range(n_ho):
                        o_sb = op_.tile([128, NT], F32, tag="o_sb")
                        nc.vector.tensor_mul(
                            o_sb[:], st["ut"][co][:, uoff + ho * NT:
                                                  uoff + (ho + 1) * NT],
                            dinv[:, ho * NT:(ho + 1) * NT])
                        nc.sync.dma_start(
                            out[co * 128:(co + 1) * 128,
                                base + ho * NT:base + (ho + 1) * NT],
                            o_sb[:])

            # ---- emission schedule -----------------------------------
            # Projection work is distributed just-in-time through pair 0's
            # key-block stream (standalone projection phases run at low PE
            # duty and HAM re-throttles). Only pair-0 prerequisites up front.
            for co in range(CK):
                for nto in range(2):
                    emit_zt(co, nto)
            for mb in range(4):     # needs only params + xa piece0; bridges
                emit_v(mb)          # the PE over the xb piece0 DMA wait

            # just-in-time jobs sprinkled through pair 0's steps
            extra = {}

            def add_extra(step, fn):
                extra.setdefault(step, []).append(fn)

            for mb in range(4, 8):          # remaining early v blocks
                add_extra(1 + (mb - 4) // 2, lambda mb=mb: emit_v(mb))
            for mb in range(8, N_MB):       # v block 4 steps ahead of its U
                add_extra(mb - 4, lambda mb=mb: emit_v(mb))
            for p in range(1, 4):           # zt pieces 1-3 anywhere in pair 0
                for i, (co, nto) in enumerate(
                        ((0, 0), (0, 1), (1, 0), (1, 1))):
                    add_extra(1 + p * 7 + i,
                              lambda co=co, nt=2 * p + nto: emit_zt(co, nt))

            attn_begin(0)
            for step in range(N_MB + SKEW):
                attn_step(0, step)
                for fn in extra.pop(step, ()):
                    fn()
            for sg in range(1, N_SEG):
                attn_begin(sg)
                for step in range(N_MB + SKEW):
                    attn_step(sg, step)
                    if step == 2:
                        emit_d(sg - 1)
                    if step == 3:
                        attn_end(sg - 1)
                    if sg == N_SEG - 1 and step == N_MB + 1:
                        # dacc complete (mb30 add emitted at this step; mb31
                        # rides in via matmul) — D/recip overlap final U MMs
                        emit_d(sg)
            attn_end(N_SEG - 1)
    nc.compile()
    return nc


def _get_nc():
    global _NC_CACHE
    if _NC_CACHE is None:
        _NC_CACHE = _build()
    return _NC_CACHE


def _w_layout(w, dtype=np.float16):
    # lhsT chunks for "x @ w.T": w_l[p, ki*C + m] = w.T[ki*128 + p, m]
    wt = np.ascontiguousarray(np.asarray(w, np.float32).T)      # [C_in, C_out]
    return np.ascontiguousarray(
        wt.reshape(CK, 128, C).transpose(1, 0, 2).reshape(128, CK * C)
    ).astype(dtype)


def kernel(x1, x2, Wq, bq, Wk, bk, Wv, bv):
    global LAST_RESULT
    x1 = np.asarray(x1, dtype=np.float32)
    x2 = np.asarray(x2, dtype=np.float32)
    Wq = np.asarray(Wq, np.float32)
    Wk = np.asarray(Wk, np.float32)
    bq = np.asarray(bq, np.float32)
    bk = np.asarray(bk, np.float32)

    Wqk = (Wq.T.astype(np.float64) @ Wk.astype(np.float64)).astype(np.float32)
    wk_bq = (Wk.T.astype(np.float64) @ bq.astype(np.float64)).astype(np.float32)
    const = float(np.dot(bq.astype(np.float64), bk.astype(np.float64)))

    x1h = x1.reshape(B, C, N).astype(np.float16)
    x2h = x2.reshape(B, C, N).astype(np.float16)

    wqk_l = _w_layout(Wqk.T)                # ZT = Wqk^T @ X  <=>  z = t @ Wqk
    wv_l = _w_layout(Wv)
    bv_rep = np.tile(np.asarray(bv, np.float32).reshape(1, C), (128, 1))
    in_maps = []
    for core in range(8):
        b, s = core % B, core // B
        xsh, xoh = (x1h, x2h) if s == 0 else (x2h, x1h)
        xb_f32 = xoh[b].astype(np.float32)
        wvec = wk_bq @ xb_f32 + const                      # [N] per key
        wb_l = np.ascontiguousarray(wvec.reshape(N_MB, 128).T)  # [128, 32] f32
        blob = np.concatenate([
            wqk_l.view(np.uint8), wv_l.view(np.uint8),
            wb_l.view(np.uint8), bv_rep.view(np.uint8),
        ], axis=1)
        in_maps.append({
            "xa": np.ascontiguousarray(xsh[b]),
            "xb": np.ascontiguousarray(xoh[b]),
            "params": blob.view(np.float16),
        })
    nc = _get_nc()
    res = run_bass_kernel_spmd(nc, in_maps, list(range(8)))
    LAST_RESULT = res
    x1_out = np.stack([res.results[b]["o"].reshape(C, H, W) for b in range(B)])
    x2_out = np.stack([res.results[B + b]["o"].reshape(C, H, W) for b in range(B)])
    return (x1_out, x2_out)
